# revision 43
# baseline (speedup 1.0000x reference)
"""Causal single-head attention on 8 Trainium2 NeuronCores.

Problem: x[4, 4096, 1024], Wq/Wk/Wv[1024, 64] ->
  out = softmax(causal(Q K^T / 8)) V   per batch, fp32.

Sharding: core i handles batch b = i//2 with query-chunk parity p = i%2 at
256-query granularity: core p owns global 256-chunks {2j+p : j=0..7}. Both
cores of a pair load the full x[b] (transposed on host to [C, T]) and
compute full K/V; causal work is balanced by interleaving query chunks.

All matmul operands are bf16 (fp32 accumulate in PSUM). Scores are computed
transposed (S^T[k, q]) with contraction HS=64, so two key tiles run
concurrently in the PE array via partition row-groups (base partition 0 and
64). Each flush step covers a "quad" (4 key tiles = 512 keys x 256 queries)
in one [128, 4, 256] PSUM tile: one exp ACTIVATE per quad, and causality is
one constant-mask bf16 multiply on the diagonal quad per slot. V carries a
ones column so softmax row-sums accumulate with P@V; the unnormalized
out^T [65, q] is DMA'd out and division + transpose happen on host.
"""

import numpy as np
import ml_dtypes

import concourse.bacc as bacc
import concourse.mybir as mybir
import concourse.tile as tile
from concourse.bass_utils import run_bass_kernel_spmd

# Problem dims
B, T, C, HS = 4, 4096, 1024, 64
P = 128           # partitions
CH = 512          # projection chunk width
CHP = 2 * CH      # chunk-pair width (one DMA)
CHA = 256         # attention query-slot width
NSLOT = 8         # query slots per core (256 wide)
CSUB = C // P     # 8 contraction subtiles
NKT = T // P      # 32 key tiles total
NPAIR = NKT // 2  # 16 key-tile pairs

BF16 = mybir.dt.bfloat16
# key tile (within quad) held by each st/pt slot: slot s <-> tile 4w+QORD[s]
QORD = (1, 3, 0, 2)


def _build_program():
    nc = bacc.Bacc("TRN2")
    f32 = mybir.dt.float32
    EXP = mybir.ActivationFunctionType.Exp

    xT = nc.dram_tensor("xT", [C, T], BF16, kind="ExternalInput").ap()
    x0 = nc.dram_tensor("x0", [P, CSUB, CH], BF16, kind="ExternalInput").ap()
    wqk = nc.dram_tensor("wqk", [C, 2 * HS], BF16, kind="ExternalInput").ap()
    wv = nc.dram_tensor("wv", [C, HS], BF16, kind="ExternalInput").ap()
    maskq_d = nc.dram_tensor("maskq", [P, 4, CHA], BF16, kind="ExternalInput").ap()
    pmask_d = nc.dram_tensor("pmask", [HS, CHA], mybir.dt.uint8, kind="ExternalInput").ap()
    out_d = nc.dram_tensor("out", [HS + 1, NSLOT * CHA], f32, kind="ExternalOutput").ap()

    xT_r = xT.rearrange("(co ci) t -> ci co t", ci=P)      # [128, 8, 4096]
    wqk_r = wqk.rearrange("(co ci) m -> ci co m", ci=P)    # [128, 8, 128]
    wv_r = wv.rearrange("(co ci) m -> ci co m", ci=P)      # [128, 8, 64]

    with tile.TileContext(nc) as tc:
        with (
            tc.tile_pool(name="const", bufs=1) as const_pool,
            tc.tile_pool(name="persist", bufs=1) as persist,
            tc.tile_pool(name="xin", bufs=3) as xpool,
            tc.tile_pool(name="x0in", bufs=1) as x0pool,
            tc.tile_pool(name="pt", bufs=4) as pt_pool,
            tc.tile_pool(name="osb", bufs=2) as osb_pool,
            tc.tile_pool(name="proj_ps", bufs=2, space="PSUM") as proj_ps,
            tc.tile_pool(name="st_ps", bufs=2, space="PSUM") as st_ps,
            tc.tile_pool(name="ot_ps", bufs=2, space="PSUM") as ot_ps,
        ):
            # const loads go on the (otherwise idle) GpSimd queue so they
            # don't serialize ahead of the first x chunk on the Sync queue
            wqk_sb = const_pool.tile([P, CSUB, 2 * HS], BF16)
            wv_sb = const_pool.tile([P, CSUB, HS], BF16)
            maskq_sb = const_pool.tile([P, 4, CHA], BF16)
            pmask_sb = const_pool.tile([HS, CHA], mybir.dt.uint8)
            # wqk gates the very first matmul: issue it on the Activation
            # queue, which is idle until the first exp (~15us in)
            nc.scalar.dma_start(wqk_sb[:], wqk_r)
            nc.gpsimd.dma_start(wv_sb[:], wv_r)
            nc.gpsimd.dma_start(maskq_sb[:], maskq_d)
            nc.gpsimd.dma_start(pmask_sb[:], pmask_d)

            # K^T pairs: [0:64, u, :] = tile 2u, [64:128, u, :] = tile 2u+1
            kt_all = persist.tile([P, NPAIR, P], BF16)
            # Q^T, pair-major: slot s at [:, s//2, (s%2)*256:(s%2+1)*256]
            qt_slot = persist.tile([P, NSLOT // 2, 2 * CHA], BF16)
            v_all = persist.tile([P, NKT, HS + 1], BF16)     # V with ones col
            nc.vector.memset(
                v_all[:, :, HS : HS + 1].bitcast(mybir.dt.uint16), 0x3F80
            )

            # first chunk arrives as 8 per-cs slices so the first projection
            # matmul only waits for 128 KB, not the full chunk
            x0_sb = x0pool.tile([P, CSUB, CH], BF16, tag="x0")
            for cs in range(CSUB):
                nc.sync.dma_start(x0_sb[:, cs, :], x0[:, cs, :])

            for cp in range(4):  # x chunk pairs
                xc = xpool.tile([P, CSUB, CHP], BF16, tag="xc")
                if cp == 0:
                    # chunk 0 comes from x0: load only chunk 1's half, and
                    # per-cs so its projection can consume slices as they land
                    for cs in range(CSUB):
                        nc.sync.dma_start(
                            xc[:, cs, CH:CHP], xT_r[:, cs, CH:CHP]
                        )
                elif cp == 1:
                    # stream in parallel with x0/chunk-1 on the Sync queue;
                    # the Activation queue is idle until the first exp
                    nc.scalar.dma_start(
                        xc[:], xT_r[:, :, cp * CHP : (cp + 1) * CHP]
                    )
                else:
                    nc.sync.dma_start(xc[:], xT_r[:, :, cp * CHP : (cp + 1) * CHP])

                for half in range(2):
                    s = 2 * cp + half  # projection chunk = slot index
                    lo = half * CH
                    xsrc = x0_sb if s == 0 else xc
                    xlo = 0 if s == 0 else lo
                    # Q^T (rows 0:64) and K^T (rows 64:128), stacked
                    qk_ps = proj_ps.tile([P, CH], f32, tag="proj")
                    for cs in range(CSUB):
                        nc.tensor.matmul(
                            qk_ps[:],
                            lhsT=wqk_sb[:, cs, :],
                            rhs=xsrc[:, cs, xlo : xlo + CH],
                            start=(cs == 0),
                            stop=(cs == CSUB - 1),
                        )
                    # chunk s holds key tiles 4s..4s+3 = pairs 2s, 2s+1
                    ksrc = qk_ps[HS:P, :].rearrange(
                        "p (i par c) -> p i par c", i=2, par=2, c=P
                    )
                    nc.vector.tensor_copy(
                        kt_all[0:HS, 2 * s : 2 * s + 2, :], ksrc[:, :, 0, :]
                    )
                    nc.vector.tensor_copy(
                        kt_all[HS:P, 2 * s : 2 * s + 2, :], ksrc[:, :, 1, :]
                    )
                    # slot s owns 256-queries [512 s + 256 p, +256): select
                    # the matching half of this chunk's Q via the predicate
                    qlo = (s % 2) * CHA
                    for hb in (0, HS):
                        nc.vector.tensor_copy(
                            qt_slot[hb : hb + HS, s // 2, qlo : qlo + CHA],
                            qk_ps[0:HS, 0:CHA],
                        )
                        nc.vector.copy_predicated(
                            qt_slot[hb : hb + HS, s // 2, qlo : qlo + CHA],
                            pmask_sb[:],
                            qk_ps[0:HS, CHA:CH],
                        )

                    # V natural ([t, h]) via x^T blocks as stationary operand
                    v_ps = proj_ps.tile([P, 4, HS], f32, tag="proj")
                    for tt in range(4):
                        for cs in range(CSUB):
                            nc.tensor.matmul(
                                v_ps[:, tt, :],
                                lhsT=xsrc[:, cs, xlo + tt * P : xlo + (tt + 1) * P],
                                rhs=wv_sb[:, cs, :],
                                start=(cs == 0),
                                stop=(cs == CSUB - 1),
                            )
                    nc.vector.tensor_copy(
                        v_all[:, 4 * s : 4 * s + 4, 0:HS], v_ps[:]
                    )

                    # flush slot j = s: quads w = 0..j, each = key
                    # tiles 4w..4w+3 vs this slot's 256 queries
                    j = s
                    qlo2 = (s % 2) * CHA
                    ot = ot_ps.tile([P, CHA], f32, tag="ot")
                    for w in range(j + 1):
                        st = st_ps.tile([P, 4, CHA], f32, tag="st")
                        # issue order alternates row groups for LDW overlap;
                        # concurrent matmuls (issues 0&1, 2&3) must hit
                        # DIFFERENT PSUM banks (a start=True bank-clear
                        # racing a concurrent drain corrupts the bank);
                        # start=True only on each bank's first write
                        for issue, (slot, o) in enumerate(
                            ((0, 1), (2, 0), (1, 3), (3, 2))
                        ):
                            u, hi = divmod(o, 2)  # pair 2w+u, row half hi
                            hb = HS if hi else 0
                            nc.tensor.matmul(
                                st[:, slot, :],
                                lhsT=kt_all[hb : hb + HS, 2 * w + u, :],
                                rhs=qt_slot[hb : hb + HS, s // 2, qlo2 : qlo2 + CHA],
                                start=(issue < 2),
                                stop=(issue >= 2),
                                skip_group_check=True,
                            )
                        pt = pt_pool.tile([P, 4, CHA], BF16, tag="pt")
                        nc.scalar.activation(
                            pt[:], st[:], EXP, scale=float(HS) ** -0.5
                        )
                        if w == j:  # diagonal quad: constant causal mask
                            nc.vector.tensor_mul(pt[:], pt[:], maskq_sb[:])
                        for slot, o in ((0, 1), (2, 0), (1, 3), (3, 2)):
                            nc.tensor.matmul(
                                ot[0 : HS + 1, :],
                                lhsT=v_all[:, 4 * w + o, :],
                                rhs=pt[:, slot, :],
                                start=(w == 0 and slot == 0),
                                stop=(w == j and slot == 3),
                            )

                    # store unnormalized out^T + sums row; host finishes
                    o_sb = osb_pool.tile([HS + 1, CHA], f32, tag="osb")
                    nc.vector.tensor_copy(o_sb[:], ot[0 : HS + 1, :])
                    # out stores on the GpSimd queue: keeps them from
                    # queueing ahead of later x loads on Sync
                    nc.gpsimd.dma_start(out_d[:, j * CHA : (j + 1) * CHA], o_sb[:])

    nc.compile()
    return nc


_CACHE = {}


def _get_program():
    if "nc" not in _CACHE:
        _CACHE["nc"] = _build_program()
    return _CACHE["nc"]


def _host_inputs(x, Wk, Wq, Wv):
    bf = ml_dtypes.bfloat16
    x = np.asarray(x, dtype=np.float32)
    wqk = np.ascontiguousarray(
        np.concatenate([np.asarray(Wq), np.asarray(Wk)], axis=1), dtype=np.float32
    ).astype(bf)
    wv = np.ascontiguousarray(np.asarray(Wv), dtype=np.float32).astype(bf)

    xTs, x0s = [], []
    for b in range(B):
        xT = np.ascontiguousarray(x[b].T).astype(bf)   # [C, T]
        xTs.append(xT)
        v = xT[:, 0:CH].reshape(CSUB, P, CH)
        x0s.append(np.ascontiguousarray(v.transpose(1, 0, 2)))

    # maskq[i, q, c] = 1 iff c >= 128*QORD[q] + i - 256 p   (diagonal quad)
    ii = np.arange(P)[:, None, None]
    qq = np.array(QORD)[None, :, None]
    cc = np.arange(CHA)[None, None, :]
    maskqs = [
        (cc >= (128 * qq + ii - 256 * p)).astype(bf) for p in range(2)
    ]
    pmasks = [np.full((HS, CHA), p, dtype=np.uint8) for p in range(2)]

    in_maps = []
    for core in range(2 * B):
        b, p = core // 2, core % 2
        in_maps.append(
            {
                "xT": xTs[b],
                "x0": x0s[b],
                "wqk": wqk,
                "wv": wv,
                "maskq": maskqs[p],
                "pmask": pmasks[p],
            }
        )
    return in_maps


def _assemble(results):
    out = np.empty((B, T, HS), dtype=np.float32)
    for core in range(2 * B):
        b, p = core // 2, core % 2
        oc = np.asarray(results[core]["out"], dtype=np.float32)  # [65, 2048]
        for j in range(NSLOT):
            g = 2 * j + p
            blk = oc[:, j * CHA : (j + 1) * CHA]
            out[b, g * CHA : (g + 1) * CHA, :] = (blk[0:HS] / blk[HS : HS + 1]).T
    return out


def run(x, Wk, Wq, Wv, trace=False):
    nc = _get_program()
    in_maps = _host_inputs(x, Wk, Wq, Wv)
    res = run_bass_kernel_spmd(nc, in_maps, list(range(2 * B)), trace=trace)
    return _assemble(res.results), res


def kernel(x, Wk, Wq, Wv):
    out, _ = run(x, Wk, Wq, Wv)
    return out
